# revision 2
# baseline (speedup 1.0000x reference)
"""CorrelationLayer Trainium2 kernel v2 (8-core SPMD, data-parallel over batch).

Per core (2 images). Pipeline:
  - f2 is loaded once per image in 1024-pixel chunks, normalized on the fly
    (ACT Square -> PE ones-matmul partition-reduce (broadcast rows) -> ACT
    Sqrt -> DVE reciprocal -> DVE multiply) and stored as bf16 in padded
    16-row "group" tiles [128, 2cb, 16, 136] resident in SBUF.
  - Per band (image, 16-row strip): f1 band loaded once, re-tiled to bf16
    stationary tiles; per-pixel inv-norms via squared-stationary diag-matmul
    (lhsT = f1^2, rhs = ones column -> ss on the partition axis).
  - Band matmul per (tile wt): moving operand is the 24x16 padded f2n
    window read strided straight out of the group tiles in 3 row-segments
    (4+16+4 rows -> disjoint PSUM column ranges), bf16, accumulated over
    2 C-blocks.  PSUM band [128, 384] holds pixel p's 81 correlations at
    band position psi(p) + 16*(dy+4) + (dx+4), psi(p) = 16*lh + lw.
  - ACT evacuates PSUM -> SBUF bf16 with the per-pixel f1 inv-norm as the
    activation scale.
  - Diagonal extraction via a sheared bf16 DRAM bounce (write row p shifted
    left by psi(p); read back a [128, 137] window per tile in one DMA per
    band), then a strided DVE copy compacts/converts to [128, 16, 81] f32.
  - Output tiles (b, ht, wt, 128, 81); host reassembles (B, 81, H, W).
"""

import numpy as np

import concourse.bass as bass
import concourse.mybir as mybir
import concourse.tile as tile
from concourse.vector_clock import ScopedClock

# ---------------------------------------------------------------------------
# Problem constants (hardcoded per spec).
B, C, H, W = 16, 256, 96, 128
NCORES = 8
BL = B // NCORES          # batch per core
CB = 2                    # C blocks of 128
TH, TW = 16, 8            # f1 tile (128 pixels)
SH, SW = TH + 8, TW + 8   # f2 window 24 x 16
N = SH * SW               # 384 band columns
HT, WT = H // TH, W // TW # 6 x 16 tiles per image
WP = W + 8                # padded row length 136
JROW = 631                # sheared scratch row length (>= N + psi_max)
PSI_MAX = 16 * (TH - 1) + (TW - 1)  # 247
WIN = N - PSI_MAX         # 137 readback window
Q = 81
F32 = mybir.dt.float32
F32R = mybir.dt.float32r
BF16 = mybir.dt.bfloat16

NORM_EPS_SQ = 1e-30
CHUNK = 1024              # pixels per norm chunk (8 rows)
RPC = CHUNK // W          # rows per chunk (8)
CPG = TH // RPC           # chunks per group (2)


# ---------------------------------------------------------------------------
# Workarounds for this walrus build: at most ONE sync-wait per instruction.
def _drain_and_barrier(self, tick_clock, wait_clock):
    nc = self.nc
    drain_inst = nc.sync.drain()
    wait_clock.add_sem_waits(
        drain_inst.ins, ScopedClock({None: tick_clock.global_clock})
    )
    si = drain_inst.ins.sync_info
    if si is not None and si.on_wait and len(si.on_wait) > 1:
        waits = list(si.on_wait)
        drain_inst.ins.sync_info = mybir.SyncInfo(
            on_wait=[waits[0]], on_update=list(si.on_update or [])
        )
        for w in waits[1:]:
            n = nc.sync.nop(nofuse=True)
            n.ins.sync_info = mybir.SyncInfo(on_wait=[w], on_update=[])
    nc.all_engine_barrier()
    assert self.sems is not None
    popped = nc._tile_sem_poison_stack.pop()
    assert popped is self._sem_poison
    nc.clear_and_free_semaphores(list(self.sems.allocated().values()))
    nc.all_engine_barrier()


tile.TileContext._drain_and_barrier = _drain_and_barrier


def split_multi_waits(nc):
    """Move extra sync-waits onto same-engine nops inserted just before."""
    counter = 0
    for fn in nc.m.functions:
        for bb in fn.blocks:
            new = []
            for inst in bb.instructions:
                si = inst.sync_info
                if si is not None and si.on_wait and len(si.on_wait) > 1:
                    waits = list(si.on_wait)
                    for w in waits[:-1]:
                        counter += 1
                        nop = mybir.InstNoOp(
                            name=f"I-waitsplit-{counter}", ins=[], outs=[]
                        )
                        nop.engine = inst.engine
                        nop.sync_info = mybir.SyncInfo(on_wait=[w], on_update=[])
                        new.append(nop)
                    inst.sync_info = mybir.SyncInfo(
                        on_wait=[waits[-1]], on_update=list(si.on_update or [])
                    )
                new.append(inst)
            bb.instructions = new


# ---------------------------------------------------------------------------
def build_program(repeats: int = 1, abl: str = ""):
    # abl: comma-set of {notail, noshear, noevac, nonorm, nof1}
    A = set(abl.split(",")) if abl else set()
    nc = bass.Bass("TRN2", target_bir_lowering=False, debug=False)
    f1d = nc.declare_dram_parameter("feat1", [BL, C, H, W], F32, isOutput=False)
    f2d = nc.declare_dram_parameter("feat2", [BL, C, H, W], F32, isOutput=False)
    outd = nc.declare_dram_parameter("out", [BL, HT, WT, 128, Q], BF16, isOutput=True)

    HWPIX = H * W  # 12288

    with tile.TileContext(nc) as tc:
        with (
            tc.tile_pool(name="const", bufs=1) as cpool,
            tc.tile_pool(name="grp", bufs=4) as gpool,
            tc.tile_pool(name="raw", bufs=2) as rpool,
            tc.tile_pool(name="sq", bufs=3) as sqpool,
            tc.tile_pool(name="nrm", bufs=2) as npool,
            tc.tile_pool(name="f1", bufs=2) as fpool,
            tc.tile_pool(name="band", bufs=2) as bpool,
            tc.tile_pool(name="tail", bufs=2) as tpool,
            tc.tile_pool(name="psn", bufs=1, space="PSUM") as psn,
            tc.tile_pool(name="psb", bufs=4, space="PSUM") as psb,
            tc.tile_pool(name="pss", bufs=2, space="PSUM") as pss,
            tc.tile_pool(name="dscr", bufs=2, space="DRAM") as dpool,
        ):
            onesr = cpool.tile([128, 128], F32R)
            ones_dram = nc.inline_tensor(np.ones((128, 128), np.float32), "ones_c")
            nc.sync.dma_start(onesr[:], ones_dram.ap().bitcast(F32R))
            ones16 = cpool.tile([128, 2], BF16)
            nc.vector.memset(ones16[:], 1.0)
            epsb = cpool.tile([128, 1], F32)
            nc.vector.memset(epsb[:], NORM_EPS_SQ)
            # zero pad group (4 rows is enough; read as rows of a group)
            zpad = cpool.tile([128, CB, 4, WP], BF16)
            nc.vector.memset(zpad[:], 0.0)

            for _ in range(repeats):
                # rolling state: group tiles for the current image
                for b in range(BL):
                    grp = [None] * HT

                    def emit_group(g, b=b, grp_ref=None):
                        gt = gpool.tile([128, CB, TH, WP], BF16, tag="grp")
                        # zero the x borders
                        nc.vector.memset(gt[:, :, :, 0:4], 0.0)
                        nc.vector.memset(gt[:, :, :, W + 4:WP], 0.0)
                        raw = rpool.tile([128, CB, TH * W], F32, tag="raw")
                        nc.sync.dma_start(
                            raw[:],
                            bass.AP(
                                f2d.ap().tensor,
                                b * (C * HWPIX) + g * TH * W,
                                [[HWPIX, 128],
                                 [128 * HWPIX, CB],
                                 [1, TH * W]],
                            ),
                        )
                        for ch in range(CPG):
                            c0 = ch * CHUNK
                            ssps = psn.tile([128, CHUNK], F32, tag="ssps")
                            for cb in range(CB):
                                sq = sqpool.tile([128, CHUNK], F32R, tag="sq")
                                nc.scalar.activation(
                                    sq[:], raw[:, cb, c0:c0 + CHUNK],
                                    mybir.ActivationFunctionType.Square,
                                )
                                for hh in range(CHUNK // 512):
                                    nc.tensor.matmul(
                                        ssps[:, hh * 512:(hh + 1) * 512],
                                        onesr,
                                        sq[:, hh * 512:(hh + 1) * 512],
                                        start=(cb == 0), stop=(cb == CB - 1),
                                    )
                            inv2 = npool.tile([128, CHUNK], F32, tag="inv2")
                            nc.scalar.activation(
                                inv2[:], ssps[:],
                                mybir.ActivationFunctionType.Sqrt,
                                bias=epsb[:],
                            )
                            nc.vector.reciprocal(inv2[:], inv2[:])
                            for cb in range(CB):
                                dst = gt[:, cb, ch * RPC:(ch + 1) * RPC, 4:W + 4]
                                nc.vector.tensor_mul(
                                    dst,
                                    raw[:, cb, c0:c0 + CHUNK].rearrange(
                                        "p (r w) -> p r w", r=RPC),
                                    inv2[:].rearrange(
                                        "p (r w) -> p r w", r=RPC),
                                )
                        return gt

                    def emit_band(ht, b=b):
                        h0 = ht * TH
                        # ---- f1 load + retile + squares ----
                        f1b = fpool.tile([128, CB, TH, W], F32, tag="f1b")
                        nc.sync.dma_start(
                            f1b[:],
                            bass.AP(
                                f1d.ap().tensor,
                                b * (C * HWPIX) + h0 * W,
                                [[HWPIX, 128], [128 * HWPIX, CB],
                                 [1, TH * W]],
                            ),
                        )
                        f1s = fpool.tile([128, CB, WT, TH * TW], BF16, tag="f1s")
                        for cb in range(CB):
                            src_ap = bass.AP(
                                f1b[:].tensor,
                                f1b[:].offset + cb * (TH * W),
                                [[CB * TH * W, 128], [TW, WT], [W, TH], [1, TW]],
                            )
                            nc.vector.tensor_copy(f1s[:, cb], src_ap)
                        f1q = fpool.tile([128, CB, WT, TH * TW], BF16, tag="f1q")
                        nc.vector.tensor_mul(f1q[:], f1s[:], f1s[:])
                        # ---- per-pixel ss1 via diag matmul ----
                        ps1 = pss.tile([128, 16], F32, tag="ps1")
                        for wt in range(WT):
                            for cb in range(CB):
                                nc.tensor.matmul(
                                    ps1[:, wt:wt + 1],
                                    f1q[:, cb, wt],
                                    ones16[:, 0:1],
                                    start=(cb == 0), stop=(cb == CB - 1),
                                )
                        inv1 = npool.tile([128, 16], F32, tag="inv1")
                        nc.scalar.activation(
                            inv1[:], ps1[:],
                            mybir.ActivationFunctionType.Sqrt,
                            bias=epsb[:],
                        )
                        nc.vector.reciprocal(inv1[:], inv1[:])

                        # ---- band matmuls + evac + shear write ----
                        gm1 = grp[ht - 1][:, :, TH - 4:TH, :] if ht > 0 \
                            else zpad[:]
                        g0 = grp[ht]
                        gp1 = grp[ht + 1][:, :, 0:4, :] if ht + 1 < HT \
                            else zpad[:]
                        bsb = bpool.tile([128, WT, N], BF16, tag="bsb")
                        dsc = dpool.tile([WT * 128 * JROW], BF16, tag="dsc")
                        for wt in range(WT):
                            w0 = wt * TW
                            ps = psb.tile([128, 512], F32, tag="band")
                            # one accumulation group per PSUM bank: start
                            # only on the first matmul, stop on the last
                            segs = [
                                (ps[:, 0:4 * SW], gm1),
                                (ps[:, 4 * SW:20 * SW], g0),
                                (ps[:, 20 * SW:N], gp1),
                            ]
                            k = 0
                            for cb in range(CB):
                                for dstp, gsrc in segs:
                                    nc.tensor.matmul(
                                        dstp, f1s[:, cb, wt],
                                        gsrc[:, cb, :, w0:w0 + SW],
                                        start=(k == 0), stop=(k == 5),
                                        skip_group_check=True,
                                    )
                                    k += 1
                            if "noevac" not in A:
                                nc.scalar.activation(
                                    bsb[:, wt], ps[:, 0:N],
                                    mybir.ActivationFunctionType.Copy,
                                    scale=inv1[:, wt:wt + 1],
                                )

                        # ---- clean band write, sheared readback ----
                        # one contiguous write per band; the per-partition
                        # diagonal offset psi(p) is applied on the read side
                        # (reads don't pay the DRAM read-modify-write cost
                        # that sheared writes do)
                        nc.sync.dma_start(
                            bass.AP(dsc.tensor, dsc.offset,
                                    [[WT * N, 128], [1, WT * N]]),
                            bsb[:].rearrange("p a b -> p (a b)"),
                        )
                        # sheared full-row readback: one 11.8KB run per
                        # partition (row p read at offset psi(p)) covers all
                        # 16 tile windows; extraction is then partition-
                        # uniform.  128 big descriptors instead of 2048
                        # small ones.
                        KLEN = (WT - 1) * N + WIN  # 5897
                        rb = tpool.tile([128, KLEN], BF16, tag="rb")
                        nc.sync.dma_start(
                            rb[:],
                            bass.AP(dsc.tensor, dsc.offset,
                                    [[TW * (WT * N) + 16, TH],
                                     [(WT * N) + 1, TW], [1, KLEN]]),
                        )
                        o32 = tpool.tile([128, WT, 9, 9], BF16, tag="o32")
                        nc.vector.tensor_copy(
                            o32[:],
                            bass.AP(rb[:].tensor, rb[:].offset,
                                    [[KLEN, 128], [N, WT],
                                     [SW, 9], [1, 9]]),
                        )
                        nc.sync.dma_start(
                            bass.AP(outd.ap().tensor,
                                    (b * HT + ht) * (WT * 128 * Q),
                                    [[Q, 128], [128 * Q, WT], [1, Q]]),
                            o32[:].rearrange("p a b c -> p (a b c)"),
                        )

                    # interleave: groups run one ahead of bands
                    grp[0] = emit_group(0)
                    grp[1] = emit_group(1)
                    emit_band(0)
                    for g in range(2, HT):
                        grp[g] = emit_group(g)
                        emit_band(g - 1)
                    emit_band(HT - 1)

    split_multi_waits(nc)
    return nc


# ---------------------------------------------------------------------------
_CACHE = {}


def _get_runner():
    if "runner" not in _CACHE:
        _CACHE["runner"] = build_program(repeats=1)
    return _CACHE["runner"]


def kernel(feat1, feat2):
    from concourse.bass_utils import run_bass_kernel_spmd

    feat1 = np.asarray(feat1, dtype=np.float32)
    feat2 = np.asarray(feat2, dtype=np.float32)
    assert feat1.shape == (B, C, H, W) and feat2.shape == (B, C, H, W)

    nc = _get_runner()
    in_maps = [
        {
            "feat1": feat1[core * BL:(core + 1) * BL],
            "feat2": feat2[core * BL:(core + 1) * BL],
        }
        for core in range(NCORES)
    ]
    res = run_bass_kernel_spmd(nc, in_maps, list(range(NCORES)))

    out = np.empty((B, Q, H, W), dtype=np.float32)
    for core in range(NCORES):
        t = np.asarray(res.results[core]["out"], dtype=np.float32)
        t = t.reshape(BL, HT, WT, TH, TW, Q)
        t = t.transpose(0, 5, 1, 3, 2, 4).reshape(BL, Q, H, W)
        out[core * BL:(core + 1) * BL] = t
    return out
